# revision 15
# baseline (speedup 1.0000x reference)
"""DepthSSIM loss on Trainium2 — 8-core data-parallel Bass kernel (v2).

Math (reference-equivalent, mask normalization cancelled):
  M = blur(mask)+1e-8;  A = B(it)*M - B(i)*B(t) = s_it*M^2
  X = B(ii)*M - B(i)^2; Y = B(tt)*M - B(t)^2   (sigma^2 * M^2)
  structure = (2A + 2C3*M^2) / (2*sqrt(relu(X*Y)) + (2C3+2e-8)*M^2)
  loss = 1 - mean(structure);  C3 = (0.03*L)^2/2, L = global max via AllReduce.

Device strategy per core (4 images, [128, 4, 512] h-major tiles):
  * masked inputs inp/tgt and mb produced by gpsimd casting DMAs with
    accum_op=mult (no engine time, no fp32 staging).
  * 11-tap separable blur: banded-Toeplitz matmuls on TensorE, bf16 PSUM
    (2x faster drains); 6 fields m,i,t,ii,tt,it; the it stage-2 uses 2*g2 and
    its drain is folded into the tm product (PSUM operand).
  * L-chain + scalar AllReduce(max) emitted right after the prologues so the
    collective hides under the blur work.
  * epilogue balanced across DVE/ACT/Pool; reciprocal via the fast custom op.
"""
import numpy as np
import ml_dtypes

import concourse.bass as bass
import concourse.tile as tile
from concourse import mybir
from concourse.bass_utils import run_bass_kernel_spmd
from concourse.tile import ScopedClock as _ScopedClock

# ----------------------------------------------------------------------------
# Workaround: this walrus build rejects >1 semaphore wait per instruction.
# ----------------------------------------------------------------------------
_MAX_WAITS = 1
_orig_commit = tile.TileContext._commit_instruction


def _commit_split(self, inst, lazy_reg_writes=True):
    si = getattr(inst, "sync_info", None)
    eng = getattr(inst, "engine", None)
    if si is not None and si.on_wait and len(si.on_wait) > _MAX_WAITS and eng is not None:
        waits = list(si.on_wait)
        excess, kept = waits[:-_MAX_WAITS], waits[-_MAX_WAITS:]
        for i in range(0, len(excess), _MAX_WAITS):
            nop = mybir.InstNoOp(
                name=self.nc.get_next_instruction_name(),
                engine=eng,
                sync_info=mybir.SyncInfo(on_wait=excess[i : i + _MAX_WAITS], on_update=[]),
                bass_nofuse=True,
            )
            _orig_commit(self, nop, lazy_reg_writes)
        inst.sync_info = mybir.SyncInfo(on_wait=kept, on_update=list(si.on_update or []))
    return _orig_commit(self, inst, lazy_reg_writes)


def _split_drain_and_barrier(self, tick_clock, wait_clock):
    drain_inst = self.nc.sync.drain()
    wait_clock.add_sem_waits(drain_inst.ins, _ScopedClock({None: tick_clock.global_clock}))
    si = drain_inst.ins.sync_info
    waits = list(si.on_wait) if (si is not None and si.on_wait) else []
    if len(waits) > _MAX_WAITS:
        drain_inst.ins.sync_info = mybir.SyncInfo(
            on_wait=waits[:_MAX_WAITS], on_update=list(si.on_update or [])
        )
        rest = waits[_MAX_WAITS:]
        for i in range(0, len(rest), _MAX_WAITS):
            d2 = self.nc.sync.drain()
            d2.ins.sync_info = mybir.SyncInfo(on_wait=rest[i : i + _MAX_WAITS], on_update=[])
    self.nc.all_engine_barrier()
    assert self.sems is not None
    popped = self.nc._tile_sem_poison_stack.pop()
    assert popped is self._sem_poison
    self.nc.clear_and_free_semaphores(list(self.sems.allocated().values()))
    self.nc.all_engine_barrier()


def _apply_tile_patches():
    tile.TileContext._commit_instruction = _commit_split
    tile.TileContext._drain_and_barrier = _split_drain_and_barrier


# ---------------------------------------------------------------------------
# Problem constants (hardcoded per spec)
# ---------------------------------------------------------------------------
N_CORES = 8
B, H, W = 32, 512, 512
BPC = B // N_CORES          # images per core
KS, PAD = 11, 5
K2 = 0.03
HC = H // 128               # h chunks
WC = W // 128               # w chunks
BAND = 138                  # uniform stage-1 rhs width
ST = [0, 123, 251, 374]     # stage-1 out-column start per h-chunk

F32 = mybir.dt.float32
BF16 = mybir.dt.bfloat16
AF = mybir.ActivationFunctionType
OP = mybir.AluOpType

USE_CAST_DMA = True         # gpsimd casting DMAs (+accum mult) for prologue

_CACHED_NC = None


def _toeplitz(g):
    T = np.zeros((H, H), np.float64)
    idx = np.arange(H)
    for k in range(KS):
        off = k - PAD
        m = idx + off
        valid = (m >= 0) & (m < H)
        T[m[valid], idx[valid]] = g[k]
    return T


def _adjust_bf16_sum(g):
    """Nudge bf16-quantized taps so their fp32 sum equals sum(g) (keeps
    blur(ones)==1 exact in bf16)."""
    target = g.sum()
    gb = g.astype(ml_dtypes.bfloat16).astype(np.float64)
    for _ in range(200):
        r = target - gb.sum()
        ulps = np.spacing(np.abs(gb).astype(np.float32)).astype(np.float64) * 2 ** (23 - 7)
        if abs(r) < ulps.min() / 2:
            break
        cand = np.where(ulps <= 2 * abs(r))[0]
        if len(cand) == 0:
            break
        k = cand[np.argmax(ulps[cand])]
        gb[k] = float(np.asarray(
            np.float32(gb[k] + np.sign(r) * ulps[k]), np.float32).astype(ml_dtypes.bfloat16))
    return gb


def _rank1_factors(window):
    w2d = np.asarray(window, np.float64).reshape(KS, KS)
    u, s, vt = np.linalg.svd(w2d)
    gv = u[:, 0] * np.sqrt(s[0])
    gh = vt[0, :] * np.sqrt(s[0])
    if gv.sum() < 0:
        gv, gh = -gv, -gh
    return gv, gh


def _bf16(a):
    return np.asarray(a, np.float32).astype(ml_dtypes.bfloat16)


def _g_blocks(window):
    gv, gh = _rank1_factors(window)
    gv, gh = _adjust_bf16_sum(gv), _adjust_bf16_sum(gh)
    Tv = _toeplitz(gv)   # stage 1 (H axis)
    Tw = _toeplitz(gh)   # stage 2 (W axis)
    g1 = np.zeros((HC, 128, BAND), np.float64)
    for hc in range(HC):
        g1[hc] = Tv[128 * hc : 128 * hc + 128, ST[hc] : ST[hc] + BAND]
    g2 = np.zeros((WC, 128, BAND), np.float64)
    for m in range(WC):
        g2[m] = Tw[128 * m : 128 * m + 128, ST[m] : ST[m] + BAND]
    return _bf16(g1), _bf16(g2), _bf16(2.0 * g2)


def _build_program():
    nc = bass.Bass()
    core_ids = list(range(N_CORES))

    inp_d = nc.dram_tensor("inp", [BPC, H, W], F32, kind="ExternalInput")
    tgt_d = nc.dram_tensor("tgt", [BPC, H, W], F32, kind="ExternalInput")
    msk_d = nc.dram_tensor("msk", [BPC, H, W], F32, kind="ExternalInput")
    g1_d = nc.dram_tensor("g1", [HC, 128, BAND], BF16, kind="ExternalInput")
    g2_d = nc.dram_tensor("g2", [WC, 128, BAND], BF16, kind="ExternalInput")
    g2x2_d = nc.dram_tensor("g2x2", [WC, 128, BAND], BF16, kind="ExternalInput")
    psum_out_d = nc.dram_tensor("psum_out", [128, 1], F32, kind="ExternalOutput")
    lmax_d = nc.dram_tensor("lmax", [1, 1], F32, kind="ExternalOutput")

    SHP = [128, HC, W]      # [partition=h%128, h-chunk, w]

    with tile.TileContext(nc) as tc:
        with tc.tile_pool(name="consts", bufs=1) as consts, \
             tc.tile_pool(name="stage", bufs=2) as stage, \
             tc.tile_pool(name="fields", bufs=1) as fields, \
             tc.tile_pool(name="lfields", bufs=2) as lfields, \
             tc.tile_pool(name="vtp", bufs=1) as vtp, \
             tc.tile_pool(name="drained", bufs=1) as drained, \
             tc.tile_pool(name="scr", bufs=1) as scr, \
             tc.tile_pool(name="keep", bufs=2) as keep, \
             tc.tile_pool(name="acc", bufs=1) as acc, \
             tc.tile_pool(name="ps1", bufs=2, space="PSUM") as ps1p, \
             tc.tile_pool(name="ps2", bufs=2, space="PSUM") as ps2p, \
             tc.tile_pool(name="dram", bufs=1, space="DRAM") as dram:

            # ---- constants
            g1sb = consts.tile([128, HC, BAND], BF16)
            nc.sync.dma_start(out=g1sb[:], in_=g1_d.rearrange("c p b -> p c b"))
            g2sb = consts.tile([128, WC, BAND], BF16)
            nc.sync.dma_start(out=g2sb[:], in_=g2_d.rearrange("c p b -> p c b"))
            g2x2sb = consts.tile([128, WC, BAND], BF16)
            nc.sync.dma_start(out=g2x2sb[:], in_=g2x2_d.rearrange("c p b -> p c b"))

            eps12 = consts.tile([128, 1], F32)
            nc.vector.memset(eps12[:], 1e-12)
            Lcols = acc.tile([128, 2 * BPC], F32)
            macc = acc.tile([128, BPC], F32)

            # ===== phase 1: masked bf16 fields via casting DMAs + L inputs ==
            base_fields = []   # per image: [mb, inp, tgt]
            for b in range(BPC):
                src_i = inp_d[b].rearrange("(c p) w -> p c w", p=128)
                src_t = tgt_d[b].rearrange("(c p) w -> p c w", p=128)
                src_m = msk_d[b].rearrange("(c p) w -> p c w", p=128)
                if USE_CAST_DMA:
                    mb = fields.tile(SHP, BF16, tag=f"mb{b}")
                    nc.gpsimd.dma_start(out=mb[:], in_=src_m)
                    inp = fields.tile(SHP, BF16, tag=f"inp{b}")
                    nc.gpsimd.dma_start(out=inp[:], in_=src_i)
                    nc.vector.tensor_mul(inp[:], inp[:], mb[:])
                    tgt = fields.tile(SHP, BF16, tag=f"tgt{b}")
                    nc.gpsimd.dma_start(out=tgt[:], in_=src_t)
                    nc.vector.tensor_mul(tgt[:], tgt[:], mb[:])
                else:
                    inp_f = stage.tile(SHP, F32, tag="ldi")
                    nc.sync.dma_start(out=inp_f[:], in_=src_i)
                    tgt_f = stage.tile(SHP, F32, tag="ldt")
                    nc.sync.dma_start(out=tgt_f[:], in_=src_t)
                    msk_f = stage.tile(SHP, F32, tag="ldm")
                    nc.sync.dma_start(out=msk_f[:], in_=src_m)
                    mb = fields.tile(SHP, BF16, tag=f"mb{b}")
                    nc.scalar.copy(mb[:], msk_f[:])
                    inp = fields.tile(SHP, BF16, tag=f"inp{b}")
                    nc.vector.tensor_mul(inp[:], inp_f[:], msk_f[:])
                    tgt = fields.tile(SHP, BF16, tag=f"tgt{b}")
                    nc.vector.tensor_mul(tgt[:], tgt_f[:], msk_f[:])

                mx1 = scr.tile(SHP, BF16, tag="u")
                nc.vector.tensor_scalar(out=mx1[:], in0=inp[:], scalar1=1.0, scalar2=-1e30,
                                        op0=OP.mult, op1=OP.max,
                                        accum_out=Lcols[:, 2 * b : 2 * b + 1])
                mx2 = scr.tile(SHP, BF16, tag="u")
                nc.vector.tensor_scalar(out=mx2[:], in0=tgt[:], scalar1=1.0, scalar2=-1e30,
                                        op0=OP.mult, op1=OP.max,
                                        accum_out=Lcols[:, 2 * b + 1 : 2 * b + 2])
                base_fields.append([mb, inp, tgt])

            def emit_squares(b):
                mb, inp, tgt = base_fields[b]
                ii = lfields.tile(SHP, BF16, tag="ii", name=f"ii_{b}")
                nc.scalar.activation(ii[:], inp[:], AF.Square)
                tt = lfields.tile(SHP, BF16, tag="tt", name=f"tt_{b}")
                nc.scalar.activation(tt[:], tgt[:], AF.Square)
                it = lfields.tile(SHP, BF16, tag="it", name=f"it_{b}")
                nc.gpsimd.tensor_mul(it[:], inp[:], tgt[:])
                return [mb, inp, tgt, ii, tt, it]

            next_blur_in = emit_squares(0)

            # ===== global L (scalar AllReduce max), early =====================
            Lloc = acc.tile([128, 1], F32)
            nc.vector.tensor_reduce(Lloc[:], Lcols[:], axis=mybir.AxisListType.X, op=OP.max)
            lb_d = dram.tile([128, 1], F32)
            nc.sync.dma_start(out=lb_d[:], in_=Lloc[:])
            Lrow = acc.tile([1, 128], F32)
            nc.sync.dma_start(out=Lrow[:], in_=lb_d[:].rearrange("p one -> (one) (p)"))
            L11 = acc.tile([1, 1], F32)
            nc.vector.reduce_max(L11[:], Lrow[:], axis=mybir.AxisListType.X)
            ccin = dram.tile([1, 1], F32)
            nc.sync.dma_start(out=ccin[:], in_=L11[:])
            ccout = dram.tile([1, 1], F32)
            nc.gpsimd.collective_compute(
                "AllReduce", OP.max, replica_groups=[core_ids],
                ins=[ccin[:]], outs=[ccout[:]])
            nc.sync.dma_start(out=lmax_d[:], in_=ccout[:])
            Lbc = acc.tile([128, 1], F32)
            nc.sync.dma_start(out=Lbc[:], in_=ccout[:].to_broadcast((128, 1)))
            twoC3 = acc.tile([128, 1], F32)
            nc.scalar.activation(twoC3[:], Lbc[:], AF.Square, scale=K2)
            twoC3e = acc.tile([128, 1], F32)
            nc.vector.tensor_scalar_add(twoC3e[:], twoC3[:], 2e-8)

            # ===== phase 2: per-image blur + epilogue (squares pipelined) ===
            for b in range(BPC):
                blur_in = next_blur_in

                # ----- stage 1: vertical blur (half-PSUM tiles, pipelined) ---
                vt = vtp.tile([128, 6, WC, W], BF16, tag="vt")
                def _copy_v(out, in_):
                    nc.vector.tensor_copy(out=out, in_=in_)

                def _copy_s(out, in_):
                    nc.scalar.copy(out, in_)

                s1_eng = [_copy_v, _copy_s, _copy_v, _copy_s, _copy_v, _copy_s]
                for fi, f in enumerate(blur_in):
                    for half in range(2):
                        ps = ps1p.tile([128, 2 * W], F32, tag="ps1",
                                       name=f"ps1_{fi}_{b}_{half}")
                        for wc in (2 * half, 2 * half + 1):
                            for hc in range(HC):
                                nc.tensor.matmul(
                                    ps[:, (wc - 2 * half) * W + ST[hc] :
                                       (wc - 2 * half) * W + ST[hc] + BAND],
                                    f[:, hc, 128 * wc : 128 * (wc + 1)],
                                    g1sb[:, hc, :],
                                    start=(hc == 0), stop=(hc == HC - 1),
                                    skip_group_check=True)
                        s1_eng[fi](vt[:, fi, 2 * half : 2 * half + 2, :],
                                   ps[:].rearrange("p (c w) -> p c w", c=2))

                if b + 1 < BPC:
                    next_blur_in = emit_squares(b + 1)

                # ----- stage 2: horizontal blur; m,i,t,ii,tt then it --------
                # per-field two half tiles (hb 0-1 / hb 2-3); it halves kept
                # in PSUM and folded into tm
                drains = {}
                it_halves = []
                s2_eng = {1: _copy_s, 2: _copy_s, 3: _copy_s, 4: _copy_s}
                for fi in range(6):
                    g2use = g2x2sb if fi == 5 else g2sb
                    if fi != 5:
                        dst = drained.tile(SHP, BF16, tag=f"dr{fi}")
                        drains[fi] = dst
                    for half in range(2):
                        ps2 = ps2p.tile([128, 2 * W], F32, tag="ps2",
                                        name=f"ps2_{fi}_{b}_{half}")
                        for hb in (2 * half, 2 * half + 1):
                            for m in range(WC):
                                nc.tensor.matmul(
                                    ps2[:, (hb - 2 * half) * W + ST[m] :
                                        (hb - 2 * half) * W + ST[m] + BAND],
                                    vt[:, fi, m, 128 * hb : 128 * (hb + 1)],
                                    g2use[:, m, :],
                                    start=(m == 0), stop=(m == WC - 1),
                                    skip_group_check=True)
                        src = ps2[:].rearrange("p (c w) -> p c w", c=2)
                        if fi == 5:
                            it_halves.append(src)
                        elif fi == 0:
                            nc.scalar.activation(
                                drains[0][:, 2 * half : 2 * half + 2, :], src,
                                AF.Copy, bias=1e-8, scale=1.0)
                        else:
                            s2_eng[fi](drains[fi][:, 2 * half : 2 * half + 2, :], src)
                M, Bi, Bt, Bii, Btt = (drains[i] for i in range(5))

                # ----- epilogue part 1 (L-independent) ----------------------
                u = scr.tile(SHP, BF16, tag="u")
                nc.vector.tensor_mul(u[:], Bi[:], Bt[:])
                tm = scr.tile(SHP, BF16, tag="tm")      # = 2*B(it)*M (g2 doubled)
                nc.vector.tensor_mul(tm[:, 0:2, :], it_halves[0], M[:, 0:2, :])
                nc.vector.tensor_mul(tm[:, 2:4, :], it_halves[1], M[:, 2:4, :])
                A2 = keep.tile(SHP, BF16, tag="A2", name=f"A2_{b}")
                nc.vector.scalar_tensor_tensor(out=A2[:], in0=u[:], scalar=-2.0,
                                               in1=tm[:], op0=OP.mult, op1=OP.add)

                bi2 = scr.tile(SHP, BF16, tag="bi2")
                nc.scalar.activation(bi2[:], Bi[:], AF.Square)
                bt2 = scr.tile(SHP, BF16, tag="bt2")
                nc.scalar.activation(bt2[:], Bt[:], AF.Square)
                pool_or_v = nc.vector if b == 0 else nc.gpsimd
                xm = scr.tile(SHP, BF16, tag="xm")
                pool_or_v.tensor_mul(xm[:], Bii[:], M[:])
                ym = scr.tile(SHP, BF16, tag="ym")
                nc.gpsimd.tensor_mul(ym[:], Btt[:], M[:])
                X = scr.tile(SHP, BF16, tag="X")
                nc.vector.tensor_sub(X[:], xm[:], bi2[:])
                Y = scr.tile(SHP, BF16, tag="Y")
                pool_or_v.tensor_sub(Y[:], ym[:], bt2[:])
                XY = scr.tile(SHP, BF16, tag="XY")
                nc.vector.tensor_mul(XY[:], X[:], Y[:])
                XYr = scr.tile(SHP, BF16, tag="bi2")
                nc.vector.tensor_scalar_max(XYr[:], XY[:], 0.0)
                sq = keep.tile(SHP, BF16, tag="sq", name=f"sq_{b}")
                nc.scalar.activation(sq[:], XYr[:], AF.Sqrt, scale=4.0, bias=eps12[:])
                P = keep.tile(SHP, BF16, tag="P", name=f"P_{b}")
                nc.gpsimd.tensor_mul(P[:], M[:], M[:])

                # ----- epilogue part 2 (needs global L) ---------------------
                num2 = scr.tile(SHP, BF16, tag="xm")
                nc.vector.scalar_tensor_tensor(out=num2[:], in0=P[:], scalar=twoC3[:, 0:1],
                                               in1=A2[:], op0=OP.mult, op1=OP.add)
                den2 = scr.tile(SHP, BF16, tag="den2")
                nc.vector.scalar_tensor_tensor(out=den2[:], in0=P[:], scalar=twoC3e[:, 0:1],
                                               in1=sq[:], op0=OP.mult, op1=OP.add)
                rec = scr.tile(SHP, F32, tag="rec")
                nc.vector.reciprocal(rec[:], den2[:])
                mout = scr.tile(SHP, BF16, tag="ym")
                nc.vector.scalar_tensor_tensor(out=mout[:], in0=num2[:], scalar=1.0,
                                               in1=rec[:], op0=OP.mult, op1=OP.mult,
                                               accum_out=macc[:, b : b + 1])

            mtot = acc.tile([128, 1], F32)
            nc.vector.tensor_reduce(mtot[:], macc[:], axis=mybir.AxisListType.X, op=OP.add)
            nc.sync.dma_start(out=psum_out_d[:], in_=mtot[:])

    return nc


def _get_nc():
    global _CACHED_NC
    if _CACHED_NC is None:
        _apply_tile_patches()
        _CACHED_NC = _build_program()
    return _CACHED_NC


def make_in_maps(input, target, mask, window):
    g1, g2, g2x2 = _g_blocks(window)
    inp = np.ascontiguousarray(np.asarray(input, np.float32)[:, 0])
    tgt = np.ascontiguousarray(np.asarray(target, np.float32)[:, 0])
    msk = np.ascontiguousarray(np.asarray(mask, np.float32)[:, 0])
    in_maps = []
    for c in range(N_CORES):
        sl = slice(c * BPC, (c + 1) * BPC)
        in_maps.append({
            "inp": inp[sl], "tgt": tgt[sl], "msk": msk[sl],
            "g1": g1, "g2": g2, "g2x2": g2x2,
        })
    return in_maps


def finish(results):
    total = 0.0
    for c in range(N_CORES):
        total += float(np.asarray(results[c]["psum_out"], np.float64).sum())
    return np.float32(1.0 - total / (B * H * W))


def kernel(input, target, mask, window):
    nc = _get_nc()
    in_maps = make_in_maps(input, target, mask, window)
    res = run_bass_kernel_spmd(nc, in_maps, list(range(N_CORES)))
    return finish(res.results)


# revision 19
# speedup vs baseline: 1.2532x; 1.2532x over previous
"""DepthSSIM loss on Trainium2 — 8-core data-parallel Bass kernel (v2).

Math (reference-equivalent, mask normalization cancelled):
  M = blur(mask)+1e-8;  A = B(it)*M - B(i)*B(t) = s_it*M^2
  X = B(ii)*M - B(i)^2; Y = B(tt)*M - B(t)^2   (sigma^2 * M^2)
  structure = (2A + 2C3*M^2) / (2*sqrt(relu(X*Y)) + (2C3+2e-8)*M^2)
  loss = 1 - mean(structure);  C3 = (0.03*L)^2/2, L = global max via AllReduce.

Device strategy per core (4 images, [128, 4, 512] h-major tiles):
  * masked inputs inp/tgt and mb produced by gpsimd casting DMAs with
    accum_op=mult (no engine time, no fp32 staging).
  * 11-tap separable blur: banded-Toeplitz matmuls on TensorE, bf16 PSUM
    (2x faster drains); 6 fields m,i,t,ii,tt,it; the it stage-2 uses 2*g2 and
    its drain is folded into the tm product (PSUM operand).
  * L-chain + scalar AllReduce(max) emitted right after the prologues so the
    collective hides under the blur work.
  * epilogue balanced across DVE/ACT/Pool; reciprocal via the fast custom op.
"""
import numpy as np
import ml_dtypes

import concourse.bass as bass
import concourse.tile as tile
from concourse import mybir
from concourse.bass_utils import run_bass_kernel_spmd
from concourse.tile import ScopedClock as _ScopedClock

# ----------------------------------------------------------------------------
# Workaround: this walrus build rejects >1 semaphore wait per instruction.
# ----------------------------------------------------------------------------
_MAX_WAITS = 1
_orig_commit = tile.TileContext._commit_instruction


def _commit_split(self, inst, lazy_reg_writes=True):
    si = getattr(inst, "sync_info", None)
    eng = getattr(inst, "engine", None)
    if si is not None and si.on_wait and len(si.on_wait) > _MAX_WAITS and eng is not None:
        waits = list(si.on_wait)
        excess, kept = waits[:-_MAX_WAITS], waits[-_MAX_WAITS:]
        for i in range(0, len(excess), _MAX_WAITS):
            nop = mybir.InstNoOp(
                name=self.nc.get_next_instruction_name(),
                engine=eng,
                sync_info=mybir.SyncInfo(on_wait=excess[i : i + _MAX_WAITS], on_update=[]),
                bass_nofuse=True,
            )
            _orig_commit(self, nop, lazy_reg_writes)
        inst.sync_info = mybir.SyncInfo(on_wait=kept, on_update=list(si.on_update or []))
    return _orig_commit(self, inst, lazy_reg_writes)


def _split_drain_and_barrier(self, tick_clock, wait_clock):
    drain_inst = self.nc.sync.drain()
    wait_clock.add_sem_waits(drain_inst.ins, _ScopedClock({None: tick_clock.global_clock}))
    si = drain_inst.ins.sync_info
    waits = list(si.on_wait) if (si is not None and si.on_wait) else []
    if len(waits) > _MAX_WAITS:
        drain_inst.ins.sync_info = mybir.SyncInfo(
            on_wait=waits[:_MAX_WAITS], on_update=list(si.on_update or [])
        )
        rest = waits[_MAX_WAITS:]
        for i in range(0, len(rest), _MAX_WAITS):
            d2 = self.nc.sync.drain()
            d2.ins.sync_info = mybir.SyncInfo(on_wait=rest[i : i + _MAX_WAITS], on_update=[])
    self.nc.all_engine_barrier()
    assert self.sems is not None
    popped = self.nc._tile_sem_poison_stack.pop()
    assert popped is self._sem_poison
    self.nc.clear_and_free_semaphores(list(self.sems.allocated().values()))
    self.nc.all_engine_barrier()


def _apply_tile_patches():
    tile.TileContext._commit_instruction = _commit_split
    tile.TileContext._drain_and_barrier = _split_drain_and_barrier


# ---------------------------------------------------------------------------
# Problem constants (hardcoded per spec)
# ---------------------------------------------------------------------------
N_CORES = 8
B, H, W = 32, 512, 512
BPC = B // N_CORES          # images per core
KS, PAD = 11, 5
K2 = 0.03
HC = H // 128               # h chunks
WC = W // 128               # w chunks
BAND = 138                  # uniform stage-1 rhs width
ST = [0, 123, 251, 374]     # stage-1 out-column start per h-chunk

F32 = mybir.dt.float32
BF16 = mybir.dt.bfloat16
AF = mybir.ActivationFunctionType
OP = mybir.AluOpType

USE_CAST_DMA = False         # gpsimd casting DMAs (+accum mult) for prologue

_CACHED_NC = None


def _toeplitz(g):
    T = np.zeros((H, H), np.float64)
    idx = np.arange(H)
    for k in range(KS):
        off = k - PAD
        m = idx + off
        valid = (m >= 0) & (m < H)
        T[m[valid], idx[valid]] = g[k]
    return T


def _adjust_bf16_sum(g):
    """Nudge bf16-quantized taps so their fp32 sum equals sum(g) (keeps
    blur(ones)==1 exact in bf16)."""
    target = g.sum()
    gb = g.astype(ml_dtypes.bfloat16).astype(np.float64)
    for _ in range(200):
        r = target - gb.sum()
        ulps = np.spacing(np.abs(gb).astype(np.float32)).astype(np.float64) * 2 ** (23 - 7)
        if abs(r) < ulps.min() / 2:
            break
        cand = np.where(ulps <= 2 * abs(r))[0]
        if len(cand) == 0:
            break
        k = cand[np.argmax(ulps[cand])]
        gb[k] = float(np.asarray(
            np.float32(gb[k] + np.sign(r) * ulps[k]), np.float32).astype(ml_dtypes.bfloat16))
    return gb


def _rank1_factors(window):
    w2d = np.asarray(window, np.float64).reshape(KS, KS)
    u, s, vt = np.linalg.svd(w2d)
    gv = u[:, 0] * np.sqrt(s[0])
    gh = vt[0, :] * np.sqrt(s[0])
    if gv.sum() < 0:
        gv, gh = -gv, -gh
    return gv, gh


def _bf16(a):
    return np.asarray(a, np.float32).astype(ml_dtypes.bfloat16)


def _g_blocks(window):
    gv, gh = _rank1_factors(window)
    gv, gh = _adjust_bf16_sum(gv), _adjust_bf16_sum(gh)
    Tv = _toeplitz(gv)   # stage 1 (H axis)
    Tw = _toeplitz(gh)   # stage 2 (W axis)
    g1 = np.zeros((HC, 128, BAND), np.float64)
    for hc in range(HC):
        g1[hc] = Tv[128 * hc : 128 * hc + 128, ST[hc] : ST[hc] + BAND]
    g2 = np.zeros((WC, 128, BAND), np.float64)
    for m in range(WC):
        g2[m] = Tw[128 * m : 128 * m + 128, ST[m] : ST[m] + BAND]
    return _bf16(g1), _bf16(g2), _bf16(2.0 * g2)


def _build_program():
    nc = bass.Bass()
    core_ids = list(range(N_CORES))

    inp_d = nc.dram_tensor("inp", [BPC, H, W], F32, kind="ExternalInput")
    tgt_d = nc.dram_tensor("tgt", [BPC, H, W], F32, kind="ExternalInput")
    msk_d = nc.dram_tensor("msk", [BPC, H, W], F32, kind="ExternalInput")
    g1_d = nc.dram_tensor("g1", [HC, 128, BAND], BF16, kind="ExternalInput")
    g2_d = nc.dram_tensor("g2", [WC, 128, BAND], BF16, kind="ExternalInput")
    g2x2_d = nc.dram_tensor("g2x2", [WC, 128, BAND], BF16, kind="ExternalInput")
    psum_out_d = nc.dram_tensor("psum_out", [128, 1], F32, kind="ExternalOutput")
    lmax_d = nc.dram_tensor("lmax", [1, 1], F32, kind="ExternalOutput")

    SHP = [128, HC, W]      # [partition=h%128, h-chunk, w]

    with tile.TileContext(nc) as tc:
        with tc.tile_pool(name="consts", bufs=1) as consts, \
             tc.tile_pool(name="stage", bufs=2) as stage, \
             tc.tile_pool(name="stagem", bufs=1) as stagem, \
             tc.tile_pool(name="fields", bufs=1) as fields, \
             tc.tile_pool(name="lfields", bufs=2) as lfields, \
             tc.tile_pool(name="vtp", bufs=1) as vtp, \
             tc.tile_pool(name="drained", bufs=1) as drained, \
             tc.tile_pool(name="scr", bufs=1) as scr, \
             tc.tile_pool(name="keep", bufs=1) as keep, \
             tc.tile_pool(name="acc", bufs=1) as acc, \
             tc.tile_pool(name="ps1", bufs=2, space="PSUM") as ps1p, \
             tc.tile_pool(name="ps2", bufs=2, space="PSUM") as ps2p, \
             tc.tile_pool(name="dram", bufs=1, space="DRAM") as dram:

            # ---- constants
            g1sb = consts.tile([128, HC, BAND], BF16)
            nc.sync.dma_start(out=g1sb[:], in_=g1_d.rearrange("c p b -> p c b"))
            g2sb = consts.tile([128, WC, BAND], BF16)
            nc.sync.dma_start(out=g2sb[:], in_=g2_d.rearrange("c p b -> p c b"))
            g2x2sb = consts.tile([128, WC, BAND], BF16)
            nc.sync.dma_start(out=g2x2sb[:], in_=g2x2_d.rearrange("c p b -> p c b"))

            eps12 = consts.tile([128, 1], F32)
            nc.vector.memset(eps12[:], 1e-12)
            Lcols = acc.tile([128, 2 * BPC], F32)
            macc = acc.tile([128, BPC], F32)

            # ===== phase 1: masked bf16 fields via casting DMAs + L inputs ==
            base_fields = []   # per image: [mb, inp, tgt]
            for b in range(BPC):
                src_i = inp_d[b].rearrange("(c p) w -> p c w", p=128)
                src_t = tgt_d[b].rearrange("(c p) w -> p c w", p=128)
                src_m = msk_d[b].rearrange("(c p) w -> p c w", p=128)
                msk_f = stagem.tile(SHP, F32, tag="ldm")
                nc.sync.dma_start(out=msk_f[:], in_=src_m)
                inp_f = stage.tile(SHP, F32, tag="ldi")
                nc.sync.dma_start(out=inp_f[:], in_=src_i)
                tgt_f = stage.tile(SHP, F32, tag="ldt")
                nc.sync.dma_start(out=tgt_f[:], in_=src_t)
                mb = fields.tile(SHP, BF16, tag=f"mb{b}")
                nc.scalar.copy(mb[:], msk_f[:])
                inp = fields.tile(SHP, BF16, tag=f"inp{b}")
                nc.vector.tensor_mul(inp[:], inp_f[:], msk_f[:])
                tgt = fields.tile(SHP, BF16, tag=f"tgt{b}")
                nc.vector.tensor_mul(tgt[:], tgt_f[:], msk_f[:])

                mx1 = scr.tile(SHP, BF16, tag="u")
                nc.vector.tensor_scalar(out=mx1[:], in0=inp[:], scalar1=1.0, scalar2=-1e30,
                                        op0=OP.mult, op1=OP.max,
                                        accum_out=Lcols[:, 2 * b : 2 * b + 1])
                mx2 = scr.tile(SHP, BF16, tag="u")
                nc.vector.tensor_scalar(out=mx2[:], in0=tgt[:], scalar1=1.0, scalar2=-1e30,
                                        op0=OP.mult, op1=OP.max,
                                        accum_out=Lcols[:, 2 * b + 1 : 2 * b + 2])
                base_fields.append([mb, inp, tgt])

            def emit_squares(b):
                mb, inp, tgt = base_fields[b]
                ii = lfields.tile(SHP, BF16, tag="ii", name=f"ii_{b}")
                nc.scalar.activation(ii[:], inp[:], AF.Square)
                tt = lfields.tile(SHP, BF16, tag="tt", name=f"tt_{b}")
                nc.scalar.activation(tt[:], tgt[:], AF.Square)
                it = lfields.tile(SHP, BF16, tag="it", name=f"it_{b}")
                nc.gpsimd.tensor_mul(it[:], inp[:], tgt[:])
                return [mb, inp, tgt, ii, tt, it]

            next_blur_in = emit_squares(0)

            # ===== global L (scalar AllReduce max), early =====================
            Lloc = acc.tile([128, 1], F32)
            nc.vector.tensor_reduce(Lloc[:], Lcols[:], axis=mybir.AxisListType.X, op=OP.max)
            lb_d = dram.tile([128, 1], F32)
            nc.sync.dma_start(out=lb_d[:], in_=Lloc[:])
            Lrow = acc.tile([1, 128], F32)
            nc.sync.dma_start(out=Lrow[:], in_=lb_d[:].rearrange("p one -> (one) (p)"))
            L11 = acc.tile([1, 1], F32)
            nc.vector.reduce_max(L11[:], Lrow[:], axis=mybir.AxisListType.X)
            ccin = dram.tile([1, 1], F32)
            nc.sync.dma_start(out=ccin[:], in_=L11[:])
            ccout = dram.tile([1, 1], F32)
            nc.gpsimd.collective_compute(
                "AllReduce", OP.max, replica_groups=[core_ids],
                ins=[ccin[:]], outs=[ccout[:]])
            nc.sync.dma_start(out=lmax_d[:], in_=ccout[:])
            Lbc = acc.tile([128, 1], F32)
            nc.sync.dma_start(out=Lbc[:], in_=ccout[:].to_broadcast((128, 1)))
            twoC3 = acc.tile([128, 1], F32)
            nc.scalar.activation(twoC3[:], Lbc[:], AF.Square, scale=K2)
            twoC3e = acc.tile([128, 1], F32)
            nc.vector.tensor_scalar_add(twoC3e[:], twoC3[:], 2e-8)

            # ===== phase 2: per-image blur + epilogue (squares pipelined) ===
            for b in range(BPC):
                blur_in = next_blur_in

                # ----- stage 1: vertical blur (half-PSUM tiles, pipelined) ---
                vt = vtp.tile([128, 6, WC, W], BF16, tag="vt")
                def _copy_v(out, in_):
                    nc.vector.tensor_copy(out=out, in_=in_)

                def _copy_s(out, in_):
                    nc.scalar.copy(out, in_)

                s1_eng = [_copy_v, _copy_s, _copy_v, _copy_s, _copy_v, _copy_s]
                for fi, f in enumerate(blur_in):
                    for half in range(2):
                        ps = ps1p.tile([128, 2 * W], F32, tag="ps1",
                                       name=f"ps1_{fi}_{b}_{half}")
                        for wc in (2 * half, 2 * half + 1):
                            for hc in range(HC):
                                nc.tensor.matmul(
                                    ps[:, (wc - 2 * half) * W + ST[hc] :
                                       (wc - 2 * half) * W + ST[hc] + BAND],
                                    f[:, hc, 128 * wc : 128 * (wc + 1)],
                                    g1sb[:, hc, :],
                                    start=(hc == 0), stop=(hc == HC - 1),
                                    skip_group_check=True)
                        s1_eng[fi](vt[:, fi, 2 * half : 2 * half + 2, :],
                                   ps[:].rearrange("p (c w) -> p c w", c=2))

                if b + 1 < BPC:
                    next_blur_in = emit_squares(b + 1)

                # ----- stage 2: horizontal blur; m,i,t,ii,tt then it --------
                # per-field two half tiles (hb 0-1 / hb 2-3); it halves kept
                # in PSUM and folded into tm
                drains = {}
                it_halves = []
                s2_eng = {1: _copy_s, 2: _copy_s, 3: _copy_s, 4: _copy_s}
                for fi in range(6):
                    g2use = g2x2sb if fi == 5 else g2sb
                    if fi != 5:
                        dst = drained.tile(SHP, BF16, tag=f"dr{fi}")
                        drains[fi] = dst
                    for half in range(2):
                        ps2 = ps2p.tile([128, 2 * W], F32, tag="ps2",
                                        name=f"ps2_{fi}_{b}_{half}")
                        for hb in (2 * half, 2 * half + 1):
                            for m in range(WC):
                                nc.tensor.matmul(
                                    ps2[:, (hb - 2 * half) * W + ST[m] :
                                        (hb - 2 * half) * W + ST[m] + BAND],
                                    vt[:, fi, m, 128 * hb : 128 * (hb + 1)],
                                    g2use[:, m, :],
                                    start=(m == 0), stop=(m == WC - 1),
                                    skip_group_check=True)
                        src = ps2[:].rearrange("p (c w) -> p c w", c=2)
                        if fi == 5:
                            it_halves.append(src)
                        elif fi == 0:
                            nc.scalar.activation(
                                drains[0][:, 2 * half : 2 * half + 2, :], src,
                                AF.Copy, bias=1e-8, scale=1.0)
                        else:
                            s2_eng[fi](drains[fi][:, 2 * half : 2 * half + 2, :], src)
                M, Bi, Bt, Bii, Btt = (drains[i] for i in range(5))

                # ----- epilogue part 1 (L-independent) ----------------------
                u = scr.tile(SHP, BF16, tag="u")
                nc.vector.tensor_mul(u[:], Bi[:], Bt[:])
                tm = scr.tile(SHP, BF16, tag="tm")      # = 2*B(it)*M (g2 doubled)
                nc.vector.tensor_mul(tm[:, 0:2, :], it_halves[0], M[:, 0:2, :])
                nc.vector.tensor_mul(tm[:, 2:4, :], it_halves[1], M[:, 2:4, :])
                A2 = keep.tile(SHP, BF16, tag="A2", name=f"A2_{b}")
                nc.vector.scalar_tensor_tensor(out=A2[:], in0=u[:], scalar=-2.0,
                                               in1=tm[:], op0=OP.mult, op1=OP.add)

                bi2 = scr.tile(SHP, BF16, tag="bi2")
                nc.scalar.activation(bi2[:], Bi[:], AF.Square)
                bt2 = scr.tile(SHP, BF16, tag="bt2")
                nc.scalar.activation(bt2[:], Bt[:], AF.Square)
                pool_or_v = nc.vector if b == 0 else nc.gpsimd
                xm = scr.tile(SHP, BF16, tag="xm")
                pool_or_v.tensor_mul(xm[:], Bii[:], M[:])
                ym = scr.tile(SHP, BF16, tag="ym")
                nc.gpsimd.tensor_mul(ym[:], Btt[:], M[:])
                X = scr.tile(SHP, BF16, tag="X")
                nc.vector.tensor_sub(X[:], xm[:], bi2[:])
                Y = scr.tile(SHP, BF16, tag="Y")
                pool_or_v.tensor_sub(Y[:], ym[:], bt2[:])
                XY = scr.tile(SHP, BF16, tag="u")
                nc.vector.tensor_mul(XY[:], X[:], Y[:])
                z = scr.tile(SHP, BF16, tag="bi2")    # 4*relu(XY)
                nc.vector.tensor_scalar(out=z[:], in0=XY[:], scalar1=0.0, scalar2=4.0,
                                        op0=OP.max, op1=OP.mult)
                arz = scr.tile(SHP, BF16, tag="X")
                nc.scalar.activation(arz[:], z[:], AF.Abs_reciprocal_sqrt, bias=eps12[:])
                sq = keep.tile(SHP, BF16, tag="sq", name=f"sq_{b}")
                nc.vector.tensor_mul(sq[:], z[:], arz[:])   # = sqrt(z) = 2*sigma_i*sigma_t*M^2
                P = keep.tile(SHP, BF16, tag="P", name=f"P_{b}")
                nc.gpsimd.tensor_mul(P[:], M[:], M[:])

                # ----- epilogue part 2 (needs global L) ---------------------
                num2 = scr.tile(SHP, BF16, tag="xm")
                nc.vector.scalar_tensor_tensor(out=num2[:], in0=P[:], scalar=twoC3[:, 0:1],
                                               in1=A2[:], op0=OP.mult, op1=OP.add)
                den2 = scr.tile(SHP, BF16, tag="tm")
                nc.vector.scalar_tensor_tensor(out=den2[:], in0=P[:], scalar=twoC3e[:, 0:1],
                                               in1=sq[:], op0=OP.mult, op1=OP.add)
                ard = scr.tile(SHP, BF16, tag="bt2")
                nc.scalar.activation(ard[:], den2[:], AF.Abs_reciprocal_sqrt, bias=eps12[:])
                rec = scr.tile(SHP, BF16, tag="u")
                nc.scalar.activation(rec[:], ard[:], AF.Square)
                mout = scr.tile(SHP, BF16, tag="ym")
                nc.vector.scalar_tensor_tensor(out=mout[:], in0=num2[:], scalar=1.0,
                                               in1=rec[:], op0=OP.mult, op1=OP.mult,
                                               accum_out=macc[:, b : b + 1])

            mtot = acc.tile([128, 1], F32)
            nc.vector.tensor_reduce(mtot[:], macc[:], axis=mybir.AxisListType.X, op=OP.add)
            nc.sync.dma_start(out=psum_out_d[:], in_=mtot[:])

    return nc


def _get_nc():
    global _CACHED_NC
    if _CACHED_NC is None:
        _apply_tile_patches()
        _CACHED_NC = _build_program()
    return _CACHED_NC


def make_in_maps(input, target, mask, window):
    g1, g2, g2x2 = _g_blocks(window)
    inp = np.ascontiguousarray(np.asarray(input, np.float32)[:, 0])
    tgt = np.ascontiguousarray(np.asarray(target, np.float32)[:, 0])
    msk = np.ascontiguousarray(np.asarray(mask, np.float32)[:, 0])
    in_maps = []
    for c in range(N_CORES):
        sl = slice(c * BPC, (c + 1) * BPC)
        in_maps.append({
            "inp": inp[sl], "tgt": tgt[sl], "msk": msk[sl],
            "g1": g1, "g2": g2, "g2x2": g2x2,
        })
    return in_maps


def finish(results):
    total = 0.0
    for c in range(N_CORES):
        total += float(np.asarray(results[c]["psum_out"], np.float64).sum())
    return np.float32(1.0 - total / (B * H * W))


def kernel(input, target, mask, window):
    nc = _get_nc()
    in_maps = make_in_maps(input, target, mask, window)
    res = run_bass_kernel_spmd(nc, in_maps, list(range(N_CORES)))
    return finish(res.results)


# revision 22
# speedup vs baseline: 1.4603x; 1.1653x over previous
"""DepthSSIM loss on Trainium2 — 8-core data-parallel Bass kernel.

Math (per reference):
  inp = input*mask ; tgt = target*mask
  mw  = blur(mask) + 1e-8
  mu_i = blur(inp)/mw ; mu_t = blur(tgt)/mw
  s_i2 = blur(inp^2)/mw - mu_i^2 ; s_t2 = blur(tgt^2)/mw - mu_t^2
  s_it = blur(inp*tgt)/mw - mu_i*mu_t
  L = max(max(inp), max(tgt)); C3 = (0.03 L)^2/2
  map = (s_it + C3) / (sqrt(relu(s_i2)+1e-12) sqrt(relu(s_t2)+1e-12) + C3 + 1e-8)
  loss = 1 - mean(map)

Device strategy (per core = 4 images [512,512]):
  * 11-tap separable blur as banded-Toeplitz matmuls on TensorE (bf16 data,
    fp32 PSUM): stage 1 contracts H with the image as the stationary operand
    (output is transposed "for free"); stage 2 contracts W with the Toeplitz
    blocks stationary.  Band edges use the per-element PSUM has_written
    accumulate (overlapping N-slices / contained sub-slice accumulates).
  * inp*tgt is avoided by blurring (inp+tgt)^2: 2*b(it) = b(ss)-b(ii)-b(tt).
  * The global max L crosses cores via a scalar AllReduce(max) collective
    mid-kernel; the final mean needs only per-partition partial sums, summed
    on the host.
"""
import numpy as np
import ml_dtypes

import concourse.bass as bass
import concourse.tile as tile
from concourse import mybir
from concourse.bass_utils import run_bass_kernel_spmd
from concourse.tile import ScopedClock as _ScopedClock

# ----------------------------------------------------------------------------
# Workaround: this walrus build rejects >1 semaphore wait per instruction.
# 1) split excess waits from Tile-scheduled instructions onto wait-only NOPs
#    committed just before them on the same engine (same-engine FIFO =>
#    identical semantics);
# 2) same for the TileContext tail drain (split across consecutive SP drains).
# ----------------------------------------------------------------------------
_MAX_WAITS = 1
_orig_commit = tile.TileContext._commit_instruction


def _commit_split(self, inst, lazy_reg_writes=True):
    si = getattr(inst, "sync_info", None)
    eng = getattr(inst, "engine", None)
    if si is not None and si.on_wait and len(si.on_wait) > _MAX_WAITS and eng is not None:
        waits = list(si.on_wait)
        excess, kept = waits[:-_MAX_WAITS], waits[-_MAX_WAITS:]
        for i in range(0, len(excess), _MAX_WAITS):
            nop = mybir.InstNoOp(
                name=self.nc.get_next_instruction_name(),
                engine=eng,
                sync_info=mybir.SyncInfo(on_wait=excess[i : i + _MAX_WAITS], on_update=[]),
                bass_nofuse=True,
            )
            _orig_commit(self, nop, lazy_reg_writes)
        inst.sync_info = mybir.SyncInfo(on_wait=kept, on_update=list(si.on_update or []))
    return _orig_commit(self, inst, lazy_reg_writes)


def _split_drain_and_barrier(self, tick_clock, wait_clock):
    drain_inst = self.nc.sync.drain()
    wait_clock.add_sem_waits(drain_inst.ins, _ScopedClock({None: tick_clock.global_clock}))
    si = drain_inst.ins.sync_info
    waits = list(si.on_wait) if (si is not None and si.on_wait) else []
    if len(waits) > _MAX_WAITS:
        drain_inst.ins.sync_info = mybir.SyncInfo(
            on_wait=waits[:_MAX_WAITS], on_update=list(si.on_update or [])
        )
        rest = waits[_MAX_WAITS:]
        for i in range(0, len(rest), _MAX_WAITS):
            d2 = self.nc.sync.drain()
            d2.ins.sync_info = mybir.SyncInfo(on_wait=rest[i : i + _MAX_WAITS], on_update=[])
    self.nc.all_engine_barrier()
    assert self.sems is not None
    popped = self.nc._tile_sem_poison_stack.pop()
    assert popped is self._sem_poison
    self.nc.clear_and_free_semaphores(list(self.sems.allocated().values()))
    self.nc.all_engine_barrier()


def _apply_tile_patches():
    tile.TileContext._commit_instruction = _commit_split
    tile.TileContext._drain_and_barrier = _split_drain_and_barrier


# ---------------------------------------------------------------------------
# Problem constants (hardcoded per spec)
# ---------------------------------------------------------------------------
N_CORES = 8
B, H, W = 32, 512, 512
BPC = B // N_CORES          # images per core
KS, PAD = 11, 5
K2 = 0.03
HC = H // 128               # h chunks
WC = W // 128               # w chunks
BAND = 138                  # uniform stage-1 rhs width (133..138 padded by Toeplitz zeros)
ST = [0, 123, 251, 374]     # stage-1 out-column start per h-chunk

F32 = mybir.dt.float32
BF16 = mybir.dt.bfloat16
AF = mybir.ActivationFunctionType
OP = mybir.AluOpType

_CACHED_NC = None


def _toeplitz(g):
    """T[m, j] = g[m - j + PAD]: blurred[j] = sum_m x[m] T[m, j] (zero pad)."""
    T = np.zeros((H, H), np.float64)
    idx = np.arange(H)
    for k in range(KS):
        off = k - PAD  # m - j
        m = idx + off
        valid = (m >= 0) & (m < H)
        T[m[valid], idx[valid]] = g[k]
    return T


def _adjust_bf16_sum(g):
    """Per-tap bf16 quantization nudged (in whole ulps) so the fp32/fp64 sum
    of the quantized taps equals sum(g).  Keeps blur(ones)==1 exact in bf16,
    which the mask-normalization cancellation relies on."""
    target = g.sum()
    gb = g.astype(ml_dtypes.bfloat16).astype(np.float64)
    for _ in range(200):
        r = target - gb.sum()
        ulps = np.spacing(np.abs(gb).astype(np.float32)).astype(np.float64) * 2 ** (23 - 7)
        if abs(r) < ulps.min() / 2:
            break
        cand = np.where(ulps <= 2 * abs(r))[0]
        if len(cand) == 0:
            break
        k = cand[np.argmax(ulps[cand])]
        gb[k] = float(np.asarray(
            np.float32(gb[k] + np.sign(r) * ulps[k]), np.float32).astype(ml_dtypes.bfloat16))
    return gb


def _rank1_factors(window):
    w2d = np.asarray(window, np.float64).reshape(KS, KS)
    u, s, vt = np.linalg.svd(w2d)
    gv = u[:, 0] * np.sqrt(s[0])
    gh = vt[0, :] * np.sqrt(s[0])
    if gv.sum() < 0:
        gv, gh = -gv, -gh
    return gv, gh


def _bf16(a):
    return np.asarray(a, np.float32).astype(ml_dtypes.bfloat16)


def _g_blocks(window):
    gv, gh = _rank1_factors(window)
    gv, gh = _adjust_bf16_sum(gv), _adjust_bf16_sum(gh)
    Tv = _toeplitz(gv)   # stage 1 (H axis)
    Tw = _toeplitz(gh)   # stage 2 (W axis)
    g1 = np.zeros((HC, 128, BAND), np.float64)
    for hc in range(HC):
        g1[hc] = Tv[128 * hc : 128 * hc + 128, ST[hc] : ST[hc] + BAND]
    g2 = np.zeros((WC, 128, BAND), np.float64)
    for m in range(WC):
        g2[m] = Tw[128 * m : 128 * m + 128, ST[m] : ST[m] + BAND]
    return _bf16(g1), _bf16(g2), _bf16(2.0 * g2)


def _build_program():
    nc = bass.Bass()
    core_ids = list(range(N_CORES))

    inp_d = nc.dram_tensor("inp", [BPC, H, W], F32, kind="ExternalInput")
    tgt_d = nc.dram_tensor("tgt", [BPC, H, W], F32, kind="ExternalInput")
    msk_d = nc.dram_tensor("msk", [BPC, H, W], F32, kind="ExternalInput")
    g1_d = nc.dram_tensor("g1", [HC, 128, BAND], BF16, kind="ExternalInput")
    g2_d = nc.dram_tensor("g2", [WC, 128, BAND], BF16, kind="ExternalInput")
    g2x2_d = nc.dram_tensor("g2x2", [WC, 128, BAND], BF16, kind="ExternalInput")
    psum_out_d = nc.dram_tensor("psum_out", [128, 1], F32, kind="ExternalOutput")
    lmax_d = nc.dram_tensor("lmax", [1, 1], F32, kind="ExternalOutput")

    SHP = [128, HC, W]  # image-tile shape: partition = h%128 / w%128, chunks, free

    with tile.TileContext(nc) as tc:
        with tc.tile_pool(name="consts", bufs=1) as consts, \
             tc.tile_pool(name="stage", bufs=3) as stage, \
             tc.tile_pool(name="fields", bufs=1) as fields, \
             tc.tile_pool(name="vtp", bufs=1) as vtp, \
             tc.tile_pool(name="btp", bufs=1) as btp, \
             tc.tile_pool(name="scrb", bufs=10) as scrb, \
             tc.tile_pool(name="scrf", bufs=3) as scrf, \
             tc.tile_pool(name="keep", bufs=1) as keep, \
             tc.tile_pool(name="acc", bufs=1) as acc, \
             tc.tile_pool(name="psv", bufs=1, space="PSUM") as psv, \
             tc.tile_pool(name="psb", bufs=1, space="PSUM") as psb, \
             tc.tile_pool(name="dram", bufs=1, space="DRAM") as dram:

            # ---- constants
            g1sb = consts.tile([128, HC, BAND], BF16)
            nc.sync.dma_start(out=g1sb[:], in_=g1_d.rearrange("c p b -> p c b"))
            g2sb = consts.tile([128, WC, BAND], BF16)
            nc.sync.dma_start(out=g2sb[:], in_=g2_d.rearrange("c p b -> p c b"))
            g2x2sb = consts.tile([128, WC, BAND], BF16)
            nc.sync.dma_start(out=g2x2sb[:], in_=g2x2_d.rearrange("c p b -> p c b"))
            eps12 = consts.tile([128, 1], F32)
            nc.vector.memset(eps12[:], 1e-12)
            eps8 = consts.tile([128, 1], F32)
            nc.vector.memset(eps8[:], 1e-8)

            Lcols = acc.tile([128, 2 * BPC], F32)
            macc = acc.tile([128, BPC], F32)
            keep_np = []   # (num2, denp) per image

            for b in range(BPC):
                # ---------- load + prologue ----------
                inp_f = stage.tile(SHP, F32, tag="ld")
                nc.sync.dma_start(out=inp_f[:], in_=inp_d[b].rearrange("(c p) w -> p c w", p=128))
                tgt_f = stage.tile(SHP, F32, tag="ld")
                nc.sync.dma_start(out=tgt_f[:], in_=tgt_d[b].rearrange("(c p) w -> p c w", p=128))
                msk_f = stage.tile(SHP, F32, tag="ld")
                nc.sync.dma_start(out=msk_f[:], in_=msk_d[b].rearrange("(c p) w -> p c w", p=128))

                mb = fields.tile(SHP, BF16, tag="mb")
                nc.gpsimd.tensor_copy(out=mb[:], in_=msk_f[:])
                inp = fields.tile(SHP, BF16, tag="inp")
                nc.vector.tensor_mul(inp[:], inp_f[:], msk_f[:])
                tgt = fields.tile(SHP, BF16, tag="tgt")
                nc.vector.tensor_mul(tgt[:], tgt_f[:], msk_f[:])

                mscr = scrb.tile(SHP, BF16, tag="esc")
                nc.vector.tensor_scalar(out=mscr[:], in0=inp[:], scalar1=1.0, scalar2=-1e30,
                                        op0=OP.mult, op1=OP.max,
                                        accum_out=Lcols[:, 2 * b : 2 * b + 1])
                mscr2 = scrb.tile(SHP, BF16, tag="esc")
                nc.vector.tensor_scalar(out=mscr2[:], in0=tgt[:], scalar1=1.0, scalar2=-1e30,
                                        op0=OP.mult, op1=OP.max,
                                        accum_out=Lcols[:, 2 * b + 1 : 2 * b + 2])

                ii = fields.tile(SHP, BF16, tag="ii")
                nc.scalar.activation(ii[:], inp[:], AF.Square)
                tt = fields.tile(SHP, BF16, tag="tt")
                nc.scalar.activation(tt[:], tgt[:], AF.Square)
                it = fields.tile(SHP, BF16, tag="it")
                nc.vector.tensor_mul(it[:], inp[:], tgt[:])

                blur_in = [mb, inp, tgt, ii, tt, it]

                # ---------- stage 1: vertical blur, transposed output ----------
                # VT[w, n] = sum_h F[h, w] * Tv[h, n]
                vts = []
                for fi, f in enumerate(blur_in):
                    ps = psv.tile([128, WC * W], F32, tag="psv")  # 4 banks
                    for wc in range(WC):
                        for hc in range(HC):
                            nc.tensor.matmul(
                                ps[:, wc * W + ST[hc] : wc * W + ST[hc] + BAND],
                                f[:, hc, 128 * wc : 128 * (wc + 1)],
                                g1sb[:, hc, :],
                                start=(hc == 0), stop=(hc == HC - 1),
                                skip_group_check=True)
                    vt = vtp.tile([128, WC, W], BF16, tag=f"vt{fi}")
                    nc.scalar.copy(vt[:], ps[:].rearrange("p (c w) -> p c w", c=WC))
                    vts.append(vt)

                # ---------- stage 2: horizontal blur (mirror of stage 1) ----------
                # B[h, n] = sum_w VT[w, h] * Tw[w, n]  (original layout back)
                bts = []
                for fi in range(6):
                    vt = vts[fi]
                    ps2 = psb.tile([128, WC * W], F32, tag="psb", name=f"ps2_{fi}_{b}")
                    g2use = g2x2sb if fi == 5 else g2sb
                    for hb in range(HC):
                        for m in range(WC):
                            nc.tensor.matmul(
                                ps2[:, hb * W + ST[m] : hb * W + ST[m] + BAND],
                                vt[:, m, 128 * hb : 128 * (hb + 1)],
                                g2use[:, m, :],
                                start=(m == 0), stop=(m == WC - 1),
                                skip_group_check=True)
                    if fi == 0:
                        # mask chain stays fp32 (+1e-8 folded into the drain copy)
                        bt = btp.tile([128, HC, W], F32, tag="bt0", name=f"bt0_{b}")
                        nc.scalar.activation(bt[:], ps2[:].rearrange("p (c w) -> p c w", c=HC),
                                             AF.Copy, bias=1e-8, scale=1.0)
                    else:
                        bt = btp.tile([128, HC, W], BF16, tag=f"bt{fi}", name=f"bt{fi}_{b}")
                        if fi % 2 == 0:
                            nc.scalar.copy(bt[:], ps2[:].rearrange("p (c w) -> p c w", c=HC))
                        else:
                            nc.vector.tensor_copy(bt[:], ps2[:].rearrange("p (c w) -> p c w", c=HC))
                    bts.append(bt)

                # ---------- epilogue part 1 (L-independent, mw-free) ----------
                M, Bi, Bt, Bii, Btt, Bit2 = bts
                u = scrb.tile(SHP, BF16, tag="esc")
                nc.vector.tensor_mul(u[:], Bi[:], Bt[:])
                tm = scrb.tile(SHP, BF16, tag="esc")
                nc.vector.tensor_mul(tm[:], Bit2[:], M[:])
                A2 = keep.tile(SHP, BF16, tag=f"A2{b}")
                nc.vector.scalar_tensor_tensor(out=A2[:], in0=u[:], scalar=-2.0,
                                               in1=tm[:], op0=OP.mult, op1=OP.add)
                bi2 = scrb.tile(SHP, BF16, tag="esc")
                nc.scalar.activation(bi2[:], Bi[:], AF.Square)
                bt2 = scrb.tile(SHP, BF16, tag="esc")
                nc.scalar.activation(bt2[:], Bt[:], AF.Square)
                xm = scrb.tile(SHP, BF16, tag="esc")
                nc.gpsimd.tensor_mul(xm[:], Bii[:], M[:])
                ym = scrb.tile(SHP, BF16, tag="esc")
                nc.gpsimd.tensor_mul(ym[:], Btt[:], M[:])
                X = scrb.tile(SHP, BF16, tag="esc")
                nc.vector.tensor_sub(X[:], xm[:], bi2[:])
                Y = scrb.tile(SHP, BF16, tag="esc")
                nc.gpsimd.tensor_sub(Y[:], ym[:], bt2[:])
                XY = scrb.tile(SHP, BF16, tag="esc")
                nc.vector.tensor_mul(XY[:], X[:], Y[:])
                z = scrb.tile(SHP, BF16, tag="esc")
                nc.vector.tensor_scalar(out=z[:], in0=XY[:], scalar1=0.0, scalar2=4.0,
                                        op0=OP.max, op1=OP.mult)
                lnz = scrb.tile(SHP, BF16, tag="esc")
                nc.scalar.activation(lnz[:], z[:], AF.Ln, bias=eps12[:])
                sq = keep.tile(SHP, BF16, tag=f"sq{b}")
                nc.scalar.activation(sq[:], lnz[:], AF.Exp, scale=0.5)
                P = keep.tile(SHP, BF16, tag=f"P{b}")
                nc.gpsimd.tensor_mul(P[:], M[:], M[:])
                keep_np.append((A2, sq, P))

            # ---------- global L (scalar AllReduce max) ----------
            Lloc = acc.tile([128, 1], F32)
            nc.vector.tensor_reduce(Lloc[:], Lcols[:], axis=mybir.AxisListType.X, op=OP.max)
            lb_d = dram.tile([128, 1], F32)
            nc.sync.dma_start(out=lb_d[:], in_=Lloc[:])
            Lrow = acc.tile([1, 128], F32)
            nc.sync.dma_start(out=Lrow[:], in_=lb_d[:].rearrange("p one -> (one) (p)"))
            L11 = acc.tile([1, 1], F32)
            nc.vector.reduce_max(L11[:], Lrow[:], axis=mybir.AxisListType.X)
            ccin = dram.tile([1, 1], F32)
            nc.sync.dma_start(out=ccin[:], in_=L11[:])
            ccout = dram.tile([1, 1], F32)
            nc.gpsimd.collective_compute(
                "AllReduce", OP.max, replica_groups=[core_ids],
                ins=[ccin[:]], outs=[ccout[:]])
            nc.sync.dma_start(out=lmax_d[:], in_=ccout[:])
            Lbc = acc.tile([128, 1], F32)
            nc.sync.dma_start(out=Lbc[:], in_=ccout[:].to_broadcast((128, 1)))
            twoC3 = acc.tile([128, 1], F32)
            nc.scalar.activation(twoC3[:], Lbc[:], AF.Square, scale=K2)
            twoC3e = acc.tile([128, 1], F32)
            nc.vector.tensor_scalar_add(twoC3e[:], twoC3[:], 2e-8)

            # ---------- epilogue part 2 ----------
            for b in range(BPC):
                A2, sq, P = keep_np[b]
                num2 = scrb.tile(SHP, BF16, tag="esc")
                nc.vector.scalar_tensor_tensor(out=num2[:], in0=P[:], scalar=twoC3[:, 0:1],
                                               in1=A2[:], op0=OP.mult, op1=OP.add)
                den2 = scrb.tile(SHP, BF16, tag="esc")
                nc.vector.scalar_tensor_tensor(out=den2[:], in0=P[:], scalar=twoC3e[:, 0:1],
                                               in1=sq[:], op0=OP.mult, op1=OP.add)
                lnd = scrb.tile(SHP, BF16, tag="esc")
                nc.scalar.activation(lnd[:], den2[:], AF.Ln, bias=eps12[:])
                rec = scrb.tile(SHP, BF16, tag="esc")
                nc.scalar.activation(rec[:], lnd[:], AF.Exp, scale=-1.0)
                mout = scrb.tile(SHP, BF16, tag="esc")
                nc.vector.scalar_tensor_tensor(out=mout[:], in0=num2[:], scalar=1.0,
                                               in1=rec[:], op0=OP.mult, op1=OP.mult,
                                               accum_out=macc[:, b : b + 1])

            mtot = acc.tile([128, 1], F32)
            nc.vector.tensor_reduce(mtot[:], macc[:], axis=mybir.AxisListType.X, op=OP.add)
            nc.sync.dma_start(out=psum_out_d[:], in_=mtot[:])

    return nc


def _get_nc():
    global _CACHED_NC
    if _CACHED_NC is None:
        _apply_tile_patches()
        _CACHED_NC = _build_program()
    return _CACHED_NC


def make_in_maps(input, target, mask, window):
    g1, g2, g2x2 = _g_blocks(window)
    inp = np.ascontiguousarray(np.asarray(input, np.float32)[:, 0])
    tgt = np.ascontiguousarray(np.asarray(target, np.float32)[:, 0])
    msk = np.ascontiguousarray(np.asarray(mask, np.float32)[:, 0])
    in_maps = []
    for c in range(N_CORES):
        sl = slice(c * BPC, (c + 1) * BPC)
        in_maps.append({
            "inp": inp[sl], "tgt": tgt[sl], "msk": msk[sl],
            "g1": g1, "g2": g2, "g2x2": g2x2,
        })
    return in_maps


def finish(results):
    total = 0.0
    for c in range(N_CORES):
        total += float(np.asarray(results[c]["psum_out"], np.float64).sum())
    return np.float32(1.0 - total / (B * H * W))


def kernel(input, target, mask, window):
    nc = _get_nc()
    in_maps = make_in_maps(input, target, mask, window)
    res = run_bass_kernel_spmd(nc, in_maps, list(range(N_CORES)))
    return finish(res.results)

